# revision 1
# baseline (speedup 1.0000x reference)
"""Trainium2 Bass kernel for nn_CombinedN2NWaveletLoss.

Strategy (pure data parallel, 8 cores x 4 images):
- Each NeuronCore gets 4 images of [512,512]; image i occupies partitions
  [32i, 32i+32); partition q (within image) owns output rows [16q, 16q+16).
- Raw input rows [16q-2, 16q+18) are DMA'd per partition (2-row halos on each
  side, duplicated/fixed-up at image edges), so every op is free-dim only.
- All heavy elementwise work runs on the Vector engine in fp16 (2x mode for
  2-tensor ops, 4x for tensor_scalar); squares+sums run on the Scalar engine
  (ACT) with accum_out; per-partition partial sums land in a [128,13] f32
  tile, DMA'd out and combined on the host in float64.

Scale folding (validated in numerics_check.py): the bilinear 2x upsample
weights (0.25,0.75) are applied as (1/3, 1.0) per stage, giving stored scales
A/0.75 and g/0.5625. The conv uses RAW weights on the scaled g1, so the conv
output carries a 1/0.5625 scale; clip bounds and wavelet thresholds are
pre-scaled accordingly and the host rescales the final sums. Haar levels skip
the 0.5 factor (stored detail scale 2^j/0.5625).

Engine/ISA notes discovered the hard way:
- Every TPB instruction supports exactly ONE sync-wait; Tile sometimes emits
  more (DMA fan-in, released-zone deps, the tail drain) -> the kernel keeps
  every op's cross-engine fan-in at 1 by construction, and a post-pass splits
  any remaining multi-wait into standalone 1-wait Drains.
- scalar_tensor_tensor (STT) runs at 1x on the DVE; tensor_scalar (4x fp16)
  + tensor_tensor (2x fp16) pairs are ~2x faster -> all hot paths use them.
- ACT must never touch pool-recycled SBUF zones (it inherits released-zone
  DMA waits) -> its outputs go to dead-but-allocated gpool/persist tiles.
"""

import numpy as np

B_TOTAL = 32
N_CORES = 8
IMGS_PER_CORE = 4
H = W = 512
QP = 32            # partitions per image
RP = 16            # output rows per partition
THRESHOLD = 50.0 / 255.0
GAMMA = 2.0
WAVELET_WEIGHT = 0.05

_CACHE = {}


def _build():
    import concourse.bass as bass
    import concourse.mybir as mybir
    import concourse.tile as tile
    from contextlib import ExitStack

    dt = mybir.dt
    Alu = mybir.AluOpType
    Act = mybir.ActivationFunctionType
    F16 = dt.float16
    F32 = dt.float32

    nc = bass.Bass("TRN2", target_bir_lowering=False, debug=False,
                   num_devices=N_CORES)
    # host-staged per-partition row windows: partition p=32i+q holds x[i]
    # rows [16q-2, 16q+18) with image-edge rows duplicated (one dense DMA,
    # single producer for the tile -> minimal sync waits on consumers)
    # row 20 carries per-partition aux data (folded conv weights) in cols
    # 0:16 and zeros in cols 16+ (source for the conv zero-pad row DMAs) --
    # folding these into xs keeps the total DMA count (and thus the tail
    # drain's sync-wait count) within hardware limits.
    xsh = nc.dram_tensor("xs", [128, 22, 512], F32, kind="ExternalInput")
    outh = nc.dram_tensor("res", [128, 13], F32, kind="ExternalOutput")

    T = THRESHOLD
    SC = 1.0 / 0.5625      # stored scale of conv output (raw w on g/0.5625)
    t1, t2, t3 = T / 4 * 2 * SC, T / 2 * 4 * SC, T * 8 * SC

    with tile.TileContext(nc) as tc, ExitStack() as ctx:
        v = nc.vector
        sc = nc.scalar

        def stt(out, in0, s, in1, op0=Alu.mult, op1=Alu.add):
            v.scalar_tensor_tensor(out=out, in0=in0, scalar=s, in1=in1,
                                   op0=op0, op1=op1)

        def tt(out, in0, in1, op):
            v.tensor_tensor(out=out, in0=in0, in1=in1, op=op)

        # ---- persistent pool: accumulators, conv output, wavelet buffers ----
        pp = ctx.enter_context(tc.tile_pool(name="persist", bufs=1))
        # one tile per accumulator slot: avoids cross-engine WAW hazards on a
        # shared tile, which would add sync waits beyond the HW per-
        # instruction limit (1 for STT/TS/ACT structs)
        accs = [pp.tile([128, 1], F32, tag=f"acc{k}", name=f"acc{k}")
                for k in range(13)]
        aux = pp.tile([128, 16], F32, tag="aux")
        warma = pp.tile([128, 1], F32, tag="warma")
        warmb = pp.tile([128, 1], F32, tag="warmb")
        oute = pp.tile([128, 16, 256], F16, tag="oute")
        outo = pp.tile([128, 16, 256], F16, tag="outo")
        sw = pp.tile([128, 16, 256], F16, tag="sw")
        dw = pp.tile([128, 16, 256], F16, tag="dw")
        ll1 = pp.tile([128, 8, 256], F16, tag="ll1")
        dett = pp.tile([128, 8, 768], F16, tag="dett")
        msc2 = pp.tile([128, 8, 256], F16, tag="msc2")
        sw2 = pp.tile([128, 8, 128], F16, tag="sw2")
        dw2 = pp.tile([128, 8, 128], F16, tag="dw2")
        ll2 = pp.tile([128, 4, 128], F16, tag="ll2")
        sw3 = pp.tile([128, 4, 64], F16, tag="sw3")
        dw3 = pp.tile([128, 4, 64], F16, tag="dw3")


        with tc.tile_pool(name="gpool", bufs=1) as gp:
            A = gp.tile([128, 18, 258], F16, tag="A")
            A2 = gp.tile([128, 18, 258], F16, tag="A2")
            Bt = gp.tile([128, 16, 258], F16, tag="Bt")
            B2 = gp.tile([128, 16, 258], F16, tag="B2")
            g1e = gp.tile([128, 18, 256], F16, tag="g1e")
            g1o = gp.tile([128, 18, 256], F16, tag="g1o")
            g1oL = gp.tile([128, 18, 256], F16, tag="g1oL")
            g1eR = gp.tile([128, 18, 256], F16, tag="g1eR")
            g2e = gp.tile([128, 16, 256], F16, tag="g2e")
            g2o = gp.tile([128, 16, 256], F16, tag="g2o")

            # ---------------- load ----------------
            # column-halved: DMA of half 2 overlaps vertical upsample of
            # half 1 (the vert pass only mixes rows, never columns)
            with tc.tile_pool(name="xload", bufs=1) as xpool:
                # two separate tiles (not one tile, two DMAs): keeps the
                # range-tracked deps precise so each vert STT waits on
                # exactly one DMA lane (1-wait HW limit)
                xtA = xpool.tile([128, 21, 256], F32, tag="xtA")
                xtB = xpool.tile([128, 20, 256], F32, tag="xtB")
                nc.sync.dma_start(out=xtA[:, :, :],
                                  in_=xsh.ap()[:, 0:21, 0:256])
                nc.sync.dma_start(out=xtB[:, :, :],
                                  in_=xsh.ap()[:, 0:20, 256:512])
                # aux values live in xs row 20 (cols 0:16 -> first half DMA)
                v.tensor_copy(aux[:, :], xtA[:, 20, 0:16])
                # ACT warm-up: pre-touch the activation path (absorbs any
                # const-table load waits with 1-wait ops)
                sc.activation(out=warma[:, 0:1], in_=aux[:, 1:2], func=Act.Copy)
                sc.activation(out=warmb[:, 0:1], in_=aux[:, 2:3], func=Act.Square)

                # checkerboard views per half
                def halves(xth):
                    xv = xth[:, 0:20, :].rearrange(
                        "p (r two) (c ctwo) -> p r two c ctwo", two=2, ctwo=2)
                    return xv[:, :, 0, :, 0], xv[:, :, 1, :, 1]  # [128,10,128]

                P0A, P3A = halves(xtA)
                P0B, P3B = halves(xtB)

                # ------- vertical upsample (stored scale /0.75) -------
                # A: rows 16q-1..16q+16 (slot s = row-(16q-1)); col slot c+1=col c
                Ar = A[:, :, :].rearrange("p (r two) c -> p r two c", two=2)
                Br = Bt[:, :, :].rearrange("p (r two) c -> p r two c", two=2)
                for (P0h, P3h), (lo, hi) in (((P0A, P3A), (0, 128)),
                                             ((P0B, P3B), (128, 256))):
                    cs = slice(1 + lo, 1 + hi)
                    # even rows r=2k (slots 1,3,..17): A[2k]=p0[k-1]/3+p0[k]
                    stt(Ar[:, :, 1, cs], P0h[:, 0:9, :], 1.0 / 3.0,
                        P0h[:, 1:10, :])
                    # odd rows (slots 0,2,..16): A[2k+1]=p0[k+1]/3+p0[k]
                    stt(Ar[:, :, 0, cs], P0h[:, 1:10, :], 1.0 / 3.0,
                        P0h[:, 0:9, :])
                    # Bt: g2 rows 16q..16q+15 (slot = row-16q)
                    stt(Br[:, :, 0, cs], P3h[:, 0:8, :], 1.0 / 3.0,
                        P3h[:, 1:9, :])
                    stt(Br[:, :, 1, cs], P3h[:, 2:10, :], 1.0 / 3.0,
                        P3h[:, 1:9, :])

            # column clamp halos
            v.tensor_copy(A[:, :, 0:1], A[:, :, 1:2])
            v.tensor_copy(A[:, :, 257:258], A[:, :, 256:257])
            v.tensor_copy(Bt[:, :, 0:1], Bt[:, :, 1:2])
            v.tensor_copy(Bt[:, :, 257:258], Bt[:, :, 256:257])

            # zero A rows -1 / 512 on image-edge partitions (-> conv zero-pad
            # rows propagate through the g1* builds). q=0 partitions are
            # quadrant-aligned -> memset; q=31 partitions need DMA zeros, each
            # followed by a same-quadrant DVE "observer" copy so that no later
            # STT needs more than one sync wait (HW STT limit is 1).
            obs = gp.tile([128, 1, 2], F16, tag="obs")
            for i in range(IMGS_PER_CORE):
                v.memset(A[QP * i:QP * i + 1, 0:1, :], 0.0)
                p31 = QP * i + 31
                nc.gpsimd.dma_start(
                    out=A[p31:p31 + 1, 17:18, :],
                    in_=xsh.ap()[i:i + 1, 20:21, 128:257].bitcast(F16))
                lo = QP * i
                v.tensor_copy(obs[lo:lo + QP, 0:1, 0:1],
                              A[lo:lo + QP, 17:18, 0:1])

            # aligned shifted copies: A2[s] = A col s ; B2[s] = B col s
            v.tensor_copy(A2[:, :, 0:257], A[:, :, 1:258])
            v.tensor_copy(B2[:, :, 0:257], Bt[:, :, 1:258])
            v.memset(A2[:, :, 257:258], 0.0)
            v.memset(B2[:, :, 257:258], 0.0)

            # ------- horizontal upsample (stored scale /0.5625) -------
            # STT runs at 1x on the DVE; a 4x tensor_scalar prescale by 1/3
            # plus a 2x tensor_tensor add is ~2x faster. Prescales live in
            # the freed xt zone (DVE-only accesses there).
            with tc.tile_pool(name="pres", bufs=1) as prp:
                A3 = prp.tile([128, 18, 258], F16, tag="A3")
                A23 = prp.tile([128, 18, 258], F16, tag="A23")
                B3 = prp.tile([128, 16, 258], F16, tag="B3")
                B23 = prp.tile([128, 16, 258], F16, tag="B23")
                third = 1.0 / 3.0
                for dst, src in ((A3, A), (A23, A2), (B3, Bt), (B23, B2)):
                    v.tensor_scalar(out=dst[:, :, :], in0=src[:, :, :],
                                    scalar1=third, scalar2=None, op0=Alu.mult)
                # slot j: g1e=col 2j, g1o=col 2j+1, g1oL=col 2j-1, g1eR=col 2j+2
                tt(g1e[:, :, :], A3[:, :, 0:256], A2[:, :, 0:256], Alu.add)
                tt(g1o[:, :, :], A3[:, :, 2:258], A2[:, :, 0:256], Alu.add)
                tt(g1oL[:, :, :], A23[:, :, 0:256], A[:, :, 0:256], Alu.add)
                tt(g1eR[:, :, :], A23[:, :, 0:256], A[:, :, 2:258], Alu.add)
                tt(g2e[:, :, :], B3[:, :, 0:256], B2[:, :, 0:256], Alu.add)
                tt(g2o[:, :, :], B3[:, :, 2:258], B2[:, :, 0:256], Alu.add)

            # conv zero boundaries: cols -1 / 512
            v.memset(g1oL[:, :, 0:1], 0.0)
            v.memset(g1eR[:, :, 255:256], 0.0)

            # ---------------- conv 3x3 + clip ----------------
            # Each of the 9 taps is prescaled by its weight (4x tensor_scalar
            # on DVE or a Copy-with-scale on ACT), then summed with a 2x
            # tensor_tensor chain. ACT takes 4 taps/phase (engine balance);
            # its outputs go to dead gpool/persist tiles (never recycled
            # space, which would add a 2nd sync wait on the 1-wait ACT op).
            with tc.tile_pool(name="convp", bufs=1) as cp:
                ca = cp.tile([128, 16, 256], F16, tag="ca")
                cb = cp.tile([128, 16, 256], F16, tag="cb")
                t0 = cp.tile([128, 16, 256], F16, tag="t0")
                t1_ = cp.tile([128, 16, 256], F16, tag="t1_")

                def conv_phase(dst, cols, act_tiles):
                    terms = []
                    k = 0
                    for dy in (0, 1, 2):
                        for arr in cols:
                            terms.append((arr[:, dy:dy + 16, :],
                                          aux[:, k:k + 1]))
                            k += 1
                    # taps 4..8 on ACT (emitted first so ACT runs ahead)
                    for (term, w_ap), dead in zip(terms[4:], act_tiles):
                        sc.activation(out=dead, in_=term, func=Act.Copy,
                                      scale=w_ap)
                    # taps 0..3 prescaled on DVE (4x), interleaved with the
                    # 2x TT chain so each t0/t1 slot is consumed before its
                    # next overwrite (emission order defines dataflow)
                    prev = None
                    nchain = 0
                    for j, (term, w_ap) in enumerate(terms[:4]):
                        tp = [t0, t1_][j % 2][:, :, :]
                        v.tensor_scalar(out=tp, in0=term,
                                        scalar1=w_ap, scalar2=None,
                                        op0=Alu.mult)
                        if prev is None:
                            prev = tp
                        else:
                            cur = [ca, cb][nchain % 2][:, :, :]
                            tt(cur, prev, tp, Alu.add)
                            prev = cur
                            nchain += 1
                    for dead in act_tiles:
                        cur = [ca, cb][nchain % 2][:, :, :]
                        tt(cur, prev, dead, Alu.add)
                        prev = cur
                        nchain += 1
                    v.tensor_scalar(out=dst[:, :, :], in0=prev,
                                    scalar1=0.0, scalar2=SC,
                                    op0=Alu.max, op1=Alu.min)

                dv24 = dett[:, :, :].rearrange("p r (a c) -> p (r a) c", a=3)
                acte = [Bt[:, 0:16, 0:256], B2[:, 0:16, 0:256],
                        sw[:, :, :], dw[:, :, :], dv24[:, 0:16, :]]
                acto = [A[:, 0:16, 0:256], A2[:, 0:16, 0:256],
                        g1oL[:, 0:16, :], outo[:, :, :], dw[:, :, :]]
                conv_phase(oute, (g1oL, g1e, g1o), acte)
                conv_phase(outo, (g1e, g1o, g1eR), acto)

                # row-pass of wavelet L1 (frees oute/outo early for deps)
                tt(sw[:, :, :], oute[:, :, :], outo[:, :, :], Alu.add)
                tt(dw[:, :, :], oute[:, :, :], outo[:, :, :], Alu.subtract)

                # ---------------- N2N losses ----------------
                # ACT dummy outputs go into dead-but-allocated gpool tiles:
                # a fresh (pool-recycled) tile would add a second sync wait
                # (released-zone dep on a DMA lane ACT never observed), and
                # the ACT struct supports only one. d2/d3 overwrite g2e/g2o,
                # which are dead after the d0/d1 reads just above them.
                # (A GPSIMD version of these diffs modeled 10us SLOWER --
                # Pool tensor_tensor is ~4x DVE cost and sits on the tail.)
                pairs = [(g2e[:, :, :], oute, 0, Bt[:, :, 0:256], g1oL),
                         (g2o[:, :, :], outo, 1, B2[:, :, 0:256], g1eR),
                         (g1e[:, 1:17, :], oute, 2, g2e[:, :, :], A),
                         (g1o[:, 1:17, :], outo, 3, g2o[:, :, :], A2)]
                for gsrc, osrc, slot, dbuf, dead in pairs:
                    tt(dbuf, gsrc, osrc[:, :, :], Alu.subtract)
                    sc.activation(out=dead[:, 0:16, 0:256], in_=dbuf,
                                  func=Act.Square,
                                  accum_out=accs[slot][:, 0:1])

        # ---------------- wavelet ----------------
        def level(s_in, d_in, thr, slots, ll_out):
            # column pass (pairs of rows of s_in/d_in); the three detail
            # arrays land side by side in one tile so a single in-place ACT
            # Abs covers them (1 ACT round-trip per level instead of 3)
            sr = s_in.rearrange("p (r two) c -> p r two c", two=2)
            dr = d_in.rearrange("p (r two) c -> p r two c", two=2)
            n = sr.shape[1]
            c = sr.shape[3]
            if ll_out is not None:
                tt(ll_out, sr[:, :, 0, :], sr[:, :, 1, :], Alu.add)
            tt(dett[:, 0:n, 0:c], dr[:, :, 0, :], dr[:, :, 1, :], Alu.add)
            tt(dett[:, 0:n, c:2 * c], sr[:, :, 0, :], sr[:, :, 1, :],
               Alu.subtract)
            tt(dett[:, 0:n, 2 * c:3 * c], dr[:, :, 0, :], dr[:, :, 1, :],
               Alu.subtract)
            sc.activation(out=dett[:, 0:n, 0:3 * c],
                          in_=dett[:, 0:n, 0:3 * c], func=Act.Abs)
            for k, sl in enumerate(slots):
                v.tensor_scalar(out=msc2[:, 0:n, 0:c],
                                in0=dett[:, 0:n, k * c:(k + 1) * c],
                                scalar1=thr, scalar2=None,
                                op0=Alu.min, op1=Alu.add,
                                accum_out=accs[sl][:, 0:1])

        level(sw[:, :, :], dw[:, :, :], t1, (4, 5, 6), ll1[:, :, :])

        # level 2: row pass on ll1 (strided col reads)
        l1r = ll1[:, :, :].rearrange("p r (c two) -> p r c two", two=2)
        tt(sw2[:, :, :], l1r[:, :, :, 0], l1r[:, :, :, 1], Alu.add)
        tt(dw2[:, :, :], l1r[:, :, :, 0], l1r[:, :, :, 1], Alu.subtract)
        level(sw2[:, :, :], dw2[:, :, :], t2, (7, 8, 9), ll2[:, :, :])

        # level 3
        l2r = ll2[:, :, :].rearrange("p r (c two) -> p r c two", two=2)
        tt(sw3[:, :, :], l2r[:, :, :, 0], l2r[:, :, :, 1], Alu.add)
        tt(dw3[:, :, :], l2r[:, :, :, 0], l2r[:, :, :, 1], Alu.subtract)
        level(sw3[:, :, :], dw3[:, :, :], t3, (10, 11, 12), None)

        # ---------------- output ----------------
        # stage accumulators into one contiguous tile on DVE (1 wait per
        # copy), then a single output DMA (1 wait). Keeps total DMA count
        # <= 8 so no DMA ever needs a second (lane-credit) sync wait.
        stg = pp.tile([128, 16], F32, tag="stg")
        for k in range(13):
            v.tensor_copy(stg[:, k:k + 1], accs[k][:, 0:1])
        nc.gpsimd.dma_start(out=outh.ap(), in_=stg[:, 0:13])

    import os
    if os.environ.get("SKIP_WAIT_SPLIT"):
        return nc
    # ---- post-pass: hardware instructions support only ONE sync-wait ----
    # Tile sometimes attaches several (e.g. the kernel-tail drain waits on
    # every DMA lane). Split extras into standalone 1-wait Drain
    # instructions inserted just before the offender on the same engine.
    for f in nc.m.functions:
        for bb in f.blocks:
            i = 0
            while i < len(bb.instructions):
                ins = bb.instructions[i]
                si = getattr(ins, "sync_info", None)
                if si is not None and si.on_wait and len(si.on_wait) > 1:
                    waits = list(si.on_wait)
                    for w in waits[:-1]:
                        d = mybir.InstDrain(
                            name=nc.get_next_instruction_name(),
                            ins=[], outs=[], bass_is_fusable=False)
                        d.engine = ins.engine
                        d.sync_info = mybir.SyncInfo(on_wait=[w],
                                                     on_update=[])
                        bb.instructions.insert(i, d)
                        i += 1
                    # keep only the last wait on the original instruction
                    ins.sync_info = mybir.SyncInfo(
                        on_wait=[waits[-1]], on_update=list(si.on_update))
                i += 1

    return nc


def _get_nc():
    if "nc" not in _CACHE:
        _CACHE["nc"] = _build()
    return _CACHE["nc"]


def _host_combine(parts):
    """parts: list (per core) of [128,13] f32 partial sums -> final scalar."""
    s = np.zeros(13, dtype=np.float64)
    for p in parts:
        s += p.astype(np.float64).sum(axis=0)
    N = B_TOTAL * H * W
    rec = (s[0] + s[1]) * 0.5625 ** 2 / N
    reg = (s[2] + s[3]) * 0.5625 ** 2 / N
    wav = 0.0
    for j, base in ((1, 4), (2, 7), (3, 10)):
        Nj = B_TOTAL * (H // 2 ** j) ** 2
        lvl = (s[base] + s[base + 1] + s[base + 2]) * 0.5625 \
            / (2.0 ** j) / Nj / 3.0
        wav += (1.0 / (3 - j + 1)) * lvl
    return np.float32(rec + GAMMA * reg + WAVELET_WEIGHT * wav)


def make_in_maps(noisy_input, weight):
    x = np.ascontiguousarray(np.asarray(noisy_input, dtype=np.float32)
                             .reshape(B_TOTAL, H, W))
    wp = np.asarray(weight, dtype=np.float32).reshape(3, 3)
    aux = np.zeros((128, 16), dtype=np.float32)
    aux[:, 0:9] = wp.reshape(-1)[None, :]
    # row window per partition q: [16q-2 .. 16q+18) with edge duplication
    q = np.arange(QP)[:, None]
    rows = q * 16 + (np.arange(20)[None, :] - 2)                  # [32,20]
    rows[0, 0:2] = [0, 1]
    rows[-1, 18:20] = [510, 511]

    auxrow = np.zeros((128, 1, 512), dtype=np.float32)
    auxrow[:, 0, 0:16] = aux

    maps = []
    for c in range(N_CORES):
        xc = x[c * IMGS_PER_CORE:(c + 1) * IMGS_PER_CORE]
        xs = xc[:, rows, :].reshape(128, 20, 512)  # [4 img, 32 q, 20, 512]
        xs = np.concatenate([xs, auxrow,
                             np.zeros((128, 1, 512), np.float32)], axis=1)
        maps.append({"xs": np.ascontiguousarray(xs)})
    return maps


def kernel(noisy_input, weight):
    from concourse.bass_utils import run_bass_kernel_spmd
    nc = _get_nc()
    in_maps = make_in_maps(noisy_input, weight)
    res = run_bass_kernel_spmd(nc, in_maps, list(range(N_CORES)))
    return _host_combine([r["res"] for r in res.results])



# revision 5
# speedup vs baseline: 1.3164x; 1.3164x over previous
"""Trainium2 Bass kernel for nn_CombinedN2NWaveletLoss — PE-conv redesign.

Layout: transposed ("T"): partitions carry image COLUMNS, free dim carries
image rows. Per core: 4 images.

- Host: extracts checkerboard phases p0=(even,even), p3=(odd,odd) (the other
  two phases are never used), transposes, converts to fp16, stages the two
  128-column windows (blk0 c0..127, blk1 c128..255) with duplicated edge rows
  for the vertical-upsample clamp.
- DVE: vertical upsample along the free dim: A'[c, 2k] = (p0[k-1]/3 + p0[k]),
  A'[c, 2k+1] = (p0[k+1]/3 + p0[k]) -- stored at 4/3 x true scale; the 0.75
  is folded into every PE stationary.
- PE: horizontal upsample + 3x3 conv FUSED as banded matmuls. N_dy = K_dy @ U
  (K_dy = conv row-kernel matrix, U = bilinear 2x upsample matrix) gives, per
  output column J, a 3-tap stencil over A'-columns. M-chunks = 64 even +
  64 odd output half-columns j' in [64t, 64t+64); the needed c-band
  [64t-1, 64t+66) fits one 128-partition window: t0 -> blk0, t1 -> Wa
  (c62..189), t2 -> Wb (c126..253), t3 -> blk1. Wa/Wb are DMA-duplicated.
  3 accumulating matmuls (dy = -1,0,1 via mov free-offset) per chunk.
- g1/g2 also via PE (2-tap banded U stationaries), accumulated with a -I x out
  matmul so PSUM holds g - out directly; ACT Square+accum / DVE
  tensor_tensor_reduce produce the N2N sums without materializing diffs.
- Eviction: ACT Relu (psum->fp16) + DVE min(.,1) 4x = clip.
- Wavelet: phases are pre-split by the M-chunk layout; DMA shuffles re-align
  partitions per level; details via TT; min(|.|,thr) via fused TS
  (abs_max, min) + TS accumulate.
"""

import numpy as np

B_TOTAL = 32
N_CORES = 8
IMG = 4
H = W = 512
HC = 256
THRESHOLD = 50.0 / 255.0
GAMMA = 2.0
WAVELET_WEIGHT = 0.05
WIN = (0, 62, 126, 128)      # K-window start c per chunk t
NACC = 25                    # 8 d1, 8 d2, 9 wavelet (3 per level)

# acc slots: 0..7 d1 (img*2+half), 8..15 d2, 16..18 lvl1, 19..21 lvl2, 22..24 lvl3
_CACHE = {}


def _upsample_matrix():
    U = np.zeros((H, HC), dtype=np.float64)
    for j in range(H):
        src = (j + 0.5) / 2.0 - 0.5
        k0 = int(np.floor(src))
        frac = src - k0
        for k, wgt in ((k0, 1 - frac), (k0 + 1, frac)):
            U[j, min(max(k, 0), HC - 1)] += wgt
    return U


def _build_stats(w):
    """[128, 17*128] fp16 stationary pack: 12 conv (t,dyi), 4 g (t), 1 -I."""
    U = _upsample_matrix()
    wm = np.asarray(w, dtype=np.float64).reshape(3, 3)
    Nm = []
    for dy in (-1, 0, 1):
        K = np.zeros((H, H))
        for j in range(H):
            for dx in (-1, 0, 1):
                if 0 <= j + dx < H:
                    K[j, j + dx] = wm[dy + 1, dx + 1]
        Nm.append(K @ U)
    out = np.zeros((128, 17 * 128), dtype=np.float32)

    def brev6(q):
        return int(f"{q:06b}"[::-1], 2)

    def lanes(t):
        # bit-reversed lane->column map: keeps every wavelet level's
        # even/odd column split partition-contiguous (lanes q<32 = even u,
        # recursively), so level shuffles are plain contiguous DMAs.
        idx = np.empty(128, dtype=np.int64)
        for m in range(128):
            jp = 64 * t + brev6(m % 64)
            idx[m] = 2 * jp + (0 if m < 64 else 1)
        return idx

    k = 0
    for t in range(4):
        J = lanes(t)
        for i in range(3):
            out[:, k * 128:(k + 1) * 128] = 0.75 * Nm[i][J, WIN[t]:WIN[t] + 128].T
            k += 1
    for t in range(4):
        J = lanes(t)
        out[:, k * 128:(k + 1) * 128] = 0.75 * U[J, WIN[t]:WIN[t] + 128].T
        k += 1
    out[:, k * 128:(k + 1) * 128] = -np.eye(128)
    return out.astype(np.float16)


def _build():
    import concourse.bass as bass
    import concourse.mybir as mybir
    import concourse.tile as tile
    from contextlib import ExitStack

    dt = mybir.dt
    Alu = mybir.AluOpType
    Act = mybir.ActivationFunctionType
    F16, F32 = dt.float16, dt.float32

    T = THRESHOLD
    THR = (T / 4 * 2, T / 2 * 4, T * 8)   # stored-scale thresholds lvl 1..3

    nc = bass.Bass("TRN2", target_bir_lowering=False, debug=False,
                   num_devices=N_CORES)
    xsh = nc.dram_tensor("xs", [128, 2, 2, IMG, 258], F16, kind="ExternalInput")
    sth = nc.dram_tensor("st", [128, 17 * 128], F16, kind="ExternalInput")
    outh = nc.dram_tensor("res", [128, NACC], F32, kind="ExternalOutput")

    with tile.TileContext(nc) as tc, ExitStack() as ctx:
        v = nc.vector
        sc = nc.scalar
        pl = nc.gpsimd

        pp = ctx.enter_context(tc.tile_pool(name="persist", bufs=1))
        xst = pp.tile([128, 2, 2, IMG, 258], F16, tag="xst")
        stats = pp.tile([128, 17 * 128], F16, tag="stats")
        qt = pp.tile([128, 2, 2, IMG, 258], F16, tag="qt")
        Ag = pp.tile([128, 2, IMG, 514], F16, tag="Ag")
        Bg = pp.tile([128, 2, IMG, 514], F16, tag="Bg")
        AgW = pp.tile([128, 2, IMG, 514], F16, tag="AgW")
        BgW = pp.tile([128, 2, IMG, 514], F16, tag="BgW")
        out = pp.tile([128, IMG, 4, 512], F16, tag="out")
        Ee = pp.tile([128, IMG, 2, 512], F16, tag="Ee")
        Oo = pp.tile([128, IMG, 2, 512], F16, tag="Oo")
        sw = pp.tile([128, IMG, 2, 512], F16, tag="sw")
        dw = pp.tile([128, IMG, 2, 512], F16, tag="dw")
        ll1 = pp.tile([128, IMG, 2, 256], F16, tag="ll1")
        lh1 = pp.tile([128, IMG, 2, 256], F16, tag="lh1")
        hl1 = pp.tile([128, IMG, 2, 256], F16, tag="hl1")
        hh1 = pp.tile([128, IMG, 2, 256], F16, tag="hh1")
        E2 = pp.tile([128, IMG, 256], F16, tag="E2")
        O2 = pp.tile([128, IMG, 256], F16, tag="O2")
        sw2 = pp.tile([128, IMG, 256], F16, tag="sw2")
        dw2 = pp.tile([128, IMG, 256], F16, tag="dw2")
        ll2 = pp.tile([128, IMG, 128], F16, tag="ll2")
        lh2 = pp.tile([128, IMG, 128], F16, tag="lh2")
        hl2 = pp.tile([128, IMG, 128], F16, tag="hl2")
        hh2 = pp.tile([128, IMG, 128], F16, tag="hh2")
        E3 = pp.tile([128, IMG, 128], F16, tag="E3")
        O3 = pp.tile([128, IMG, 128], F16, tag="O3")
        sw3 = pp.tile([128, IMG, 128], F16, tag="sw3")
        dw3 = pp.tile([128, IMG, 128], F16, tag="dw3")
        lh3 = pp.tile([128, IMG, 64], F16, tag="lh3")
        hl3 = pp.tile([128, IMG, 64], F16, tag="hl3")
        hh3 = pp.tile([128, IMG, 64], F16, tag="hh3")
        acc = pp.tile([128, NACC], F32, tag="acc")
        deadA = pp.tile([128, 2, 512], F16, tag="deadA")
        deadV = pp.tile([128, 2, 512], F16, tag="deadV")
        warm = pp.tile([128, 512], F16, tag="warm")

        ppre = ctx.enter_context(tc.tile_pool(name="ppre", bufs=2, space="PSUM"))
        pg = ctx.enter_context(tc.tile_pool(name="pg", bufs=2, space="PSUM"))

        # ---------------- input DMAs ----------------
        nc.sync.dma_start(out=stats[:, :], in_=sth.ap())
        nc.sync.dma_start(out=xst[:, 0, :, :, :], in_=xsh.ap()[:, 0])
        nc.sync.dma_start(out=xst[:, 1, :, :, :], in_=xsh.ap()[:, 1])
        v.memset(acc[:, :], 0.0)
        v.memset(warm[:, :], 0.0)

        # PE warm-up: keep the tensor engine busy while inputs land
        # (borrows a rotating ppre slot; released before the first conv)
        wps = ppre.tile([128, 2, 512], F32, name="wps", tag="pre")
        for _ in range(8):
            nc.tensor.matmul(wps[:, 0, :], warm[:, 0:128], warm[:, :],
                             start=True, stop=True)

        # ---------------- vertical upsample ----------------
        # qt = xst / 3  (one 4x TS per tensor half)
        for s in range(2):
            v.tensor_scalar(out=qt[:, s], in0=xst[:, s], scalar1=1.0 / 3.0,
                            scalar2=None, op0=Alu.mult)
        # guards then evens/odds; A on DVE, B on Pool
        for g_, eng in ((Ag, v), (Bg, pl)):
            v.memset(g_[:, :, :, 0:1], 0.0)
            v.memset(g_[:, :, :, 513:514], 0.0)
        for s, g_, eng in ((0, Ag, v), (1, Bg, pl)):
            gr = g_[:, :, :, 1:513].rearrange(
                "p w m (k two) -> p w m k two", two=2)
            # even: q[k] + x[k+1] ; odd: q[k+2] + x[k+1]   (stored = true*4/3)
            eng.tensor_tensor(out=gr[:, :, :, :, 0], in0=qt[:, s, :, :, 0:256],
                              in1=xst[:, s, :, :, 1:257], op=Alu.add)
            eng.tensor_tensor(out=gr[:, :, :, :, 1], in0=qt[:, s, :, :, 2:258],
                              in1=xst[:, s, :, :, 1:257], op=Alu.add)

        # ---------------- window duplication (Wa c62..189, Wb c126..253) ---
        for src, dst in ((Ag, AgW), (Bg, BgW)):
            nc.gpsimd.dma_start(out=dst[0:66, 0], in_=src[62:128, 0])
            nc.gpsimd.dma_start(out=dst[66:128, 0], in_=src[0:62, 1])
            nc.gpsimd.dma_start(out=dst[0:2, 1], in_=src[126:128, 0])
            nc.gpsimd.dma_start(out=dst[2:128, 1], in_=src[0:126, 1])

        movs_A = (Ag, AgW, AgW, Ag)
        movs_B = (Bg, BgW, BgW, Bg)
        wsl = (0, 0, 1, 1)   # which window slot in the tile
        negI = stats[:, 16 * 128:17 * 128]

        def conv_img(m, half):
            """half in (0,1): t chunks (2*half, 2*half+1). Returns psum."""
            pre = ppre.tile([128, 2, 512], F32, name="pre", tag="pre")
            for ti in range(2):
                t = 2 * half + ti
                mv = movs_A[t]
                for dyi, dy in enumerate((-1, 0, 1)):
                    nc.tensor.matmul(
                        pre[:, ti, :],
                        stats[:, (3 * t + dyi) * 128:(3 * t + dyi + 1) * 128],
                        mv[:, wsl[t], m, 1 + dy:513 + dy],
                        start=(dyi == 0), stop=(dyi == 2))
            return pre

        def g_img(m, half, movs, d_slot, use_act):
            gp = pg.tile([128, 2, 512], F32, name="gp", tag="g")
            for ti in range(2):
                t = 2 * half + ti
                nc.tensor.matmul(gp[:, ti, :],
                                 stats[:, (12 + t) * 128:(13 + t) * 128],
                                 movs[t][:, wsl[t], m, 1:513],
                                 start=True, stop=False)
                nc.tensor.matmul(gp[:, ti, :], negI, out[:, m, t, :],
                                 start=False, stop=True)
            # HW allows only ONE psum operand per DVE op, so the d-square
            # reductions run on ACT (Square + accum straight from psum).
            dtile = deadA if use_act else deadV
            sc.activation(out=dtile[:, :, :], in_=gp[:, :, :],
                          func=Act.Square, accum_out=acc[:, d_slot:d_slot + 1])

        for m in range(IMG):
            for half in range(2):
                pre = conv_img(m, half)
                sc.activation(out=out[:, m, 2 * half:2 * half + 2, :],
                              in_=pre[:, :, :], func=Act.Relu)
                v.tensor_scalar(out=out[:, m, 2 * half:2 * half + 2, :],
                                in0=out[:, m, 2 * half:2 * half + 2, :],
                                scalar1=1.0, scalar2=None, op0=Alu.min)
                # g1 (reg) via ACT square; g2 (rec) via DVE TTR
                g_img(m, half, movs_A, m * 2 + half, True)
                g_img(m, half, movs_B, 8 + m * 2 + half, False)
            # wavelet phase shuffles for img m (even t -> lanes 0:64 of src)
            nc.sync.dma_start(out=Ee[0:64, m, :, :],
                              in_=out[0:64, m, 0::2, :])
            nc.sync.dma_start(out=Ee[64:128, m, :, :],
                              in_=out[0:64, m, 1::2, :])
            nc.sync.dma_start(out=Oo[0:64, m, :, :],
                              in_=out[64:128, m, 0::2, :])
            nc.sync.dma_start(out=Oo[64:128, m, :, :],
                              in_=out[64:128, m, 1::2, :])

        # ---------------- wavelet ----------------
        v.tensor_tensor(out=sw[:, :, :, :], in0=Ee[:, :, :, :],
                        in1=Oo[:, :, :, :], op=Alu.add)
        pl.tensor_tensor(out=dw[:, :, :, :], in0=Ee[:, :, :, :],
                         in1=Oo[:, :, :, :], op=Alu.subtract)

        # lvl1
        swr = sw[:, :, :, :].rearrange("p a b (k two) -> p a b k two", two=2)
        dwr = dw[:, :, :, :].rearrange("p a b (k two) -> p a b k two", two=2)
        v.tensor_tensor(out=ll1[:, :, :, :], in0=swr[..., 0], in1=swr[..., 1],
                        op=Alu.add)
        v.tensor_tensor(out=lh1[:, :, :, :], in0=dwr[..., 0], in1=dwr[..., 1],
                        op=Alu.add)
        v.tensor_tensor(out=hl1[:, :, :, :], in0=swr[..., 0], in1=swr[..., 1],
                        op=Alu.subtract)
        pl.tensor_tensor(out=hh1[:, :, :, :], in0=dwr[..., 0], in1=dwr[..., 1],
                         op=Alu.subtract)
        for k, det in enumerate((lh1, hl1, hh1)):
            sc.activation(out=det[:, :, :, :], in_=det[:, :, :, :],
                          func=Act.Abs)
            v.tensor_scalar(out=det[:, :, :, :], in0=det[:, :, :, :],
                            scalar1=THR[0], scalar2=None,
                            op0=Alu.min, op1=Alu.add,
                            accum_out=acc[:, 16 + k:17 + k])

        # lvl2 shuffles (bit-reversed lanes -> contiguous ranges):
        # E2[32*tv + s] <- ll1[(tv&1)*64 + s, :, tv>>1, :]      (s < 32)
        # O2[32*tv + s] <- ll1[(tv&1)*64 + 32 + s, :, tv>>1, :]
        for tv in range(4):
            sb = (tv & 1) * 64
            th = tv >> 1
            nc.gpsimd.dma_start(out=E2[32 * tv:32 * tv + 32, :, :],
                                in_=ll1[sb:sb + 32, :, th, :])
            nc.gpsimd.dma_start(out=O2[32 * tv:32 * tv + 32, :, :],
                                in_=ll1[sb + 32:sb + 64, :, th, :])
        v.tensor_tensor(out=sw2[:, :, :], in0=E2[:, :, :], in1=O2[:, :, :],
                        op=Alu.add)
        v.tensor_tensor(out=dw2[:, :, :], in0=E2[:, :, :], in1=O2[:, :, :],
                        op=Alu.subtract)
        s2r = sw2[:, :, :].rearrange("p a (k two) -> p a k two", two=2)
        d2r = dw2[:, :, :].rearrange("p a (k two) -> p a k two", two=2)
        v.tensor_tensor(out=ll2[:, :, :], in0=s2r[..., 0], in1=s2r[..., 1],
                        op=Alu.add)
        v.tensor_tensor(out=lh2[:, :, :], in0=d2r[..., 0], in1=d2r[..., 1],
                        op=Alu.add)
        v.tensor_tensor(out=hl2[:, :, :], in0=s2r[..., 0], in1=s2r[..., 1],
                        op=Alu.subtract)
        v.tensor_tensor(out=hh2[:, :, :], in0=d2r[..., 0], in1=d2r[..., 1],
                        op=Alu.subtract)
        for k, det in enumerate((lh2, hl2, hh2)):
            sc.activation(out=det[:, :, :], in_=det[:, :, :], func=Act.Abs)
            v.tensor_scalar(out=det[:, :, :], in0=det[:, :, :],
                            scalar1=THR[1], scalar2=None,
                            op0=Alu.min, op1=Alu.add,
                            accum_out=acc[:, 19 + k:20 + k])

        # lvl3: E3[16*tv + r] <- ll2[32*tv + r] (r < 16), O3 <- +16
        for tv in range(4):
            nc.gpsimd.dma_start(out=E3[16 * tv:16 * tv + 16, :, :],
                                in_=ll2[32 * tv:32 * tv + 16, :, :])
            nc.gpsimd.dma_start(out=O3[16 * tv:16 * tv + 16, :, :],
                                in_=ll2[32 * tv + 16:32 * tv + 32, :, :])
        v.tensor_tensor(out=sw3[0:64, :, :], in0=E3[0:64, :, :],
                        in1=O3[0:64, :, :], op=Alu.add)
        v.tensor_tensor(out=dw3[0:64, :, :], in0=E3[0:64, :, :],
                        in1=O3[0:64, :, :], op=Alu.subtract)
        s3r = sw3[:, :, :].rearrange("p a (k two) -> p a k two", two=2)
        d3r = dw3[:, :, :].rearrange("p a (k two) -> p a k two", two=2)
        v.tensor_tensor(out=lh3[0:64, :, :], in0=d3r[0:64, :, :, 0],
                        in1=d3r[0:64, :, :, 1], op=Alu.add)
        v.tensor_tensor(out=hl3[0:64, :, :], in0=s3r[0:64, :, :, 0],
                        in1=s3r[0:64, :, :, 1], op=Alu.subtract)
        v.tensor_tensor(out=hh3[0:64, :, :], in0=d3r[0:64, :, :, 0],
                        in1=d3r[0:64, :, :, 1], op=Alu.subtract)
        for k, det in enumerate((lh3, hl3, hh3)):
            sc.activation(out=det[0:64, :, :], in_=det[0:64, :, :],
                          func=Act.Abs)
            v.tensor_scalar(out=det[0:64, :, :], in0=det[0:64, :, :],
                            scalar1=THR[2], scalar2=None,
                            op0=Alu.min, op1=Alu.add,
                            accum_out=acc[0:64, 22 + k:23 + k])

        nc.gpsimd.dma_start(out=outh.ap(), in_=acc[:, :])

    import os
    if not os.environ.get("SKIP_WAIT_SPLIT"):
        _split_multiwaits(nc, mybir)
    return nc


def _split_multiwaits(nc, mybir):
    """HW instructions support exactly ONE sync-wait; split extras into
    standalone Drains (same post-pass as the previous kernel)."""
    for f in nc.m.functions:
        for bb in f.blocks:
            i = 0
            while i < len(bb.instructions):
                ins = bb.instructions[i]
                si = getattr(ins, "sync_info", None)
                if si is not None and si.on_wait and len(si.on_wait) > 1:
                    waits = list(si.on_wait)
                    for w in waits[:-1]:
                        d = mybir.InstDrain(
                            name=nc.get_next_instruction_name(),
                            ins=[], outs=[], bass_is_fusable=False)
                        d.engine = ins.engine
                        d.sync_info = mybir.SyncInfo(on_wait=[w], on_update=[])
                        bb.instructions.insert(i, d)
                        i += 1
                    ins.sync_info = mybir.SyncInfo(
                        on_wait=[waits[-1]], on_update=list(si.on_update))
                i += 1


def _get_nc():
    if "nc" not in _CACHE:
        _CACHE["nc"] = _build()
    return _CACHE["nc"]


def make_in_maps(noisy_input, weight):
    x = np.asarray(noisy_input, dtype=np.float32).reshape(B_TOTAL, H, W)
    stats = _build_stats(weight)
    maps = []
    for c in range(N_CORES):
        xs = np.zeros((128, 2, 2, IMG, 258), dtype=np.float16)
        for m in range(IMG):
            img = x[c * IMG + m]
            for s, ph in enumerate((img[0::2, 0::2], img[1::2, 1::2])):
                pt = np.ascontiguousarray(ph.T).astype(np.float16)  # [c, r]
                st = np.concatenate([pt[:, :1], pt, pt[:, -1:]], axis=1)
                xs[:, s, 0, m, :] = st[0:128]
                xs[:, s, 1, m, :] = st[128:256]
        maps.append({"xs": xs, "st": stats})
    return maps


def _host_combine(parts):
    d1 = d2 = 0.0
    wav = np.zeros(3)
    for p in parts:
        q = p.astype(np.float64)
        d1 += q[:, 0:8].sum()
        d2 += q[:, 8:16].sum()
        wav[0] += q[:, 16:19].sum()
        wav[1] += q[:, 19:22].sum()
        wav[2] += q[0:64, 22:25].sum()
    N = B_TOTAL * H * W
    reg = d1 / N
    rec = d2 / N
    wtot = 0.0
    for j in (1, 2, 3):
        lvl = 3 - j + 1
        Nj = B_TOTAL * (H // 2 ** j) ** 2 * 3
        wtot += (1.0 / lvl) * (wav[j - 1] / (2.0 ** j)) / Nj
    return np.float32(rec + GAMMA * reg + WAVELET_WEIGHT * wtot)


def kernel(noisy_input, weight):
    from concourse.bass_utils import run_bass_kernel_spmd
    nc = _get_nc()
    in_maps = make_in_maps(noisy_input, weight)
    res = run_bass_kernel_spmd(nc, in_maps, list(range(N_CORES)))
    return _host_combine([r["res"] for r in res.results])


# revision 9
# speedup vs baseline: 1.8756x; 1.4248x over previous
"""Trainium2 Bass kernel for nn_CombinedN2NWaveletLoss — PE-conv redesign.

Layout: transposed ("T"): partitions carry image COLUMNS, free dim carries
image rows. Per core: 4 images.

- Host: extracts checkerboard phases p0=(even,even), p3=(odd,odd) (the other
  two phases are never used), transposes, converts to fp16, stages the two
  128-column windows (blk0 c0..127, blk1 c128..255) with duplicated edge rows
  for the vertical-upsample clamp.
- DVE: vertical upsample along the free dim: A'[c, 2k] = (p0[k-1]/3 + p0[k]),
  A'[c, 2k+1] = (p0[k+1]/3 + p0[k]) -- stored at 4/3 x true scale; the 0.75
  is folded into every PE stationary.
- PE: horizontal upsample + 3x3 conv FUSED as banded matmuls. N_dy = K_dy @ U
  (K_dy = conv row-kernel matrix, U = bilinear 2x upsample matrix) gives, per
  output column J, a 3-tap stencil over A'-columns. M-chunks = 64 even +
  64 odd output half-columns j' in [64t, 64t+64); the needed c-band
  [64t-1, 64t+66) fits one 128-partition window: t0 -> blk0, t1 -> Wa
  (c62..189), t2 -> Wb (c126..253), t3 -> blk1. Wa/Wb are DMA-duplicated.
  3 accumulating matmuls (dy = -1,0,1 via mov free-offset) per chunk.
- g1/g2 also via PE (2-tap banded U stationaries), accumulated with a -I x out
  matmul so PSUM holds g - out directly; ACT Square+accum / DVE
  tensor_tensor_reduce produce the N2N sums without materializing diffs.
- Eviction: ACT Relu (psum->fp16) + DVE min(.,1) 4x = clip.
- Wavelet: phases are pre-split by the M-chunk layout; DMA shuffles re-align
  partitions per level; details via TT; min(|.|,thr) via fused TS
  (abs_max, min) + TS accumulate.
"""

import numpy as np

B_TOTAL = 32
N_CORES = 8
IMG = 4
H = W = 512
HC = 256
THRESHOLD = 50.0 / 255.0
GAMMA = 2.0
WAVELET_WEIGHT = 0.05
WIN = (0, 62, 126, 128)      # K-window start c per chunk t
NACC = 34                    # 8 d1, 8 d2, 12 lvl1 (per img), 3 lvl2, 3 lvl3

# acc slots: 0..7 d1, 8..15 d2, 16+3i+k lvl1, 28..30 lvl2, 31..33 lvl3
_CACHE = {}


def _upsample_matrix():
    U = np.zeros((H, HC), dtype=np.float64)
    for j in range(H):
        src = (j + 0.5) / 2.0 - 0.5
        k0 = int(np.floor(src))
        frac = src - k0
        for k, wgt in ((k0, 1 - frac), (k0 + 1, frac)):
            U[j, min(max(k, 0), HC - 1)] += wgt
    return U


def _build_stats(w):
    """[128, 17*128] fp16 stationary pack: 12 conv (t,dyi), 4 g (t), 1 -I."""
    U = _upsample_matrix()
    wm = np.asarray(w, dtype=np.float64).reshape(3, 3)
    Nm = []
    for dy in (-1, 0, 1):
        K = np.zeros((H, H))
        for j in range(H):
            for dx in (-1, 0, 1):
                if 0 <= j + dx < H:
                    K[j, j + dx] = wm[dy + 1, dx + 1]
        Nm.append(K @ U)
    out = np.zeros((128, 17 * 128), dtype=np.float32)

    def brev6(q):
        return int(f"{q:06b}"[::-1], 2)

    def lanes(t):
        # bit-reversed lane->column map: keeps every wavelet level's
        # even/odd column split partition-contiguous (lanes q<32 = even u,
        # recursively), so level shuffles are plain contiguous DMAs.
        idx = np.empty(128, dtype=np.int64)
        for m in range(128):
            jp = 64 * t + brev6(m % 64)
            idx[m] = 2 * jp + (0 if m < 64 else 1)
        return idx

    k = 0
    for t in range(4):
        J = lanes(t)
        for i in range(3):
            out[:, k * 128:(k + 1) * 128] = 0.75 * Nm[i][J, WIN[t]:WIN[t] + 128].T
            k += 1
    for t in range(4):
        J = lanes(t)
        out[:, k * 128:(k + 1) * 128] = 0.75 * U[J, WIN[t]:WIN[t] + 128].T
        k += 1
    out[:, k * 128:(k + 1) * 128] = -np.eye(128)
    return out.astype(np.float16)


def _build():
    import concourse.bass as bass
    import concourse.mybir as mybir
    import concourse.tile as tile
    from contextlib import ExitStack

    dt = mybir.dt
    Alu = mybir.AluOpType
    Act = mybir.ActivationFunctionType
    F16, F32 = dt.float16, dt.float32

    T = THRESHOLD
    THR = (T / 4 * 2, T / 2 * 4, T * 8)   # stored-scale thresholds lvl 1..3

    nc = bass.Bass("TRN2", target_bir_lowering=False, debug=False,
                   num_devices=N_CORES)
    xsh = nc.dram_tensor("xs", [128, 2, 2, IMG, 258], F16, kind="ExternalInput")
    sth = nc.dram_tensor("st", [128, 17 * 128], F16, kind="ExternalInput")
    outh = nc.dram_tensor("res", [128, NACC], F32, kind="ExternalOutput")

    with tile.TileContext(nc) as tc, ExitStack() as ctx:
        v = nc.vector
        sc = nc.scalar
        pl = nc.gpsimd

        pp = ctx.enter_context(tc.tile_pool(name="persist", bufs=1))
        xst = pp.tile([128, 2, 2, IMG, 258], F16, tag="xst")
        stats = pp.tile([128, 17 * 128], F16, tag="stats")
        qt = pp.tile([128, 2, 2, IMG, 258], F16, tag="qt")
        # parity planes: [..., 0, r] = A'[2r] (+guard r=256,257),
        #                [..., 1, r] = A'[2r-1] (guards r=0, 257)
        Ag = pp.tile([128, 2, IMG, 2, 258], F16, tag="Ag")
        Bg = pp.tile([128, 2, IMG, 2, 258], F16, tag="Bg")
        AgW = pp.tile([128, 2, IMG, 2, 258], F16, tag="AgW")
        BgW = pp.tile([128, 2, IMG, 2, 258], F16, tag="BgW")
        out = pp.tile([128, IMG, 4, 512], F16, tag="out")
        Ee = pp.tile([128, IMG, 2, 512], F16, tag="Ee")
        Oo = pp.tile([128, IMG, 2, 512], F16, tag="Oo")
        sw = pp.tile([128, IMG, 2, 512], F16, tag="sw")
        dw = pp.tile([128, IMG, 2, 512], F16, tag="dw")
        ll1 = pp.tile([128, IMG, 2, 256], F16, tag="ll1")
        lh1 = pp.tile([128, IMG, 2, 256], F16, tag="lh1")
        hl1 = pp.tile([128, IMG, 2, 256], F16, tag="hl1")
        hh1 = pp.tile([128, IMG, 2, 256], F16, tag="hh1")
        E2 = pp.tile([128, IMG, 256], F16, tag="E2")
        O2 = pp.tile([128, IMG, 256], F16, tag="O2")
        sw2 = pp.tile([128, IMG, 256], F16, tag="sw2")
        dw2 = pp.tile([128, IMG, 256], F16, tag="dw2")
        ll2 = pp.tile([128, IMG, 128], F16, tag="ll2")
        lh2 = pp.tile([128, IMG, 128], F16, tag="lh2")
        hl2 = pp.tile([128, IMG, 128], F16, tag="hl2")
        hh2 = pp.tile([128, IMG, 128], F16, tag="hh2")
        E3 = pp.tile([128, IMG, 128], F16, tag="E3")
        O3 = pp.tile([128, IMG, 128], F16, tag="O3")
        sw3 = pp.tile([128, IMG, 128], F16, tag="sw3")
        dw3 = pp.tile([128, IMG, 128], F16, tag="dw3")
        lh3 = pp.tile([128, IMG, 64], F16, tag="lh3")
        hl3 = pp.tile([128, IMG, 64], F16, tag="hl3")
        hh3 = pp.tile([128, IMG, 64], F16, tag="hh3")
        acc = pp.tile([128, NACC], F32, tag="acc")
        deadA = pp.tile([128, 2, 512], F16, tag="deadA")
        deadV = pp.tile([128, 2, 512], F16, tag="deadV")
        warm = pp.tile([128, 512], F16, tag="warm")

        ppre = ctx.enter_context(tc.tile_pool(name="ppre", bufs=2, space="PSUM"))
        pg = ctx.enter_context(tc.tile_pool(name="pg", bufs=2, space="PSUM"))

        # ---------------- input DMAs (fine-grained, 2 queues) ----------
        nc.scalar.dma_start(out=stats[:, :], in_=sth.ap())
        for si in range(2):
            for w in range(2):
                q = nc.sync if (si + w) % 2 == 0 else nc.gpsimd
                q.dma_start(out=xst[:, si, w, :, :], in_=xsh.ap()[:, si, w])
        v.memset(acc[:, :], 0.0)
        v.memset(warm[:, :], 0.0)

        # PE warm-up: keep the tensor engine busy while inputs land
        # (borrows a rotating ppre slot; released before the first conv)
        wps = ppre.tile([128, 2, 512], F32, name="wps", tag="pre")
        for _ in range(8):
            nc.tensor.matmul(wps[:, 0, :], warm[:, 0:128], warm[:, :],
                             start=True, stop=True)

        # ---------------- vertical upsample (parity planes, 2x) ----------
        for g_ in (Ag, Bg):
            v.memset(g_[:, :, :, 0, 256:258], 0.0)   # A'[512] guard + pad
            v.memset(g_[:, :, :, 1, 0:1], 0.0)       # A'[-1] guard
            v.memset(g_[:, :, :, 1, 257:258], 0.0)
        for si, g_ in ((0, Ag), (1, Bg)):
            for w in range(2):
                v.tensor_scalar(out=qt[:, si, w], in0=xst[:, si, w],
                                scalar1=1.0 / 3.0, scalar2=None, op0=Alu.mult)
                # A'[2k] = q[k] + x[k+1]; A'[2k+1] = q[k+2] + x[k+1]
                v.tensor_tensor(out=g_[:, w, :, 0, 0:256],
                                in0=qt[:, si, w, :, 0:256],
                                in1=xst[:, si, w, :, 1:257], op=Alu.add)
                v.tensor_tensor(out=g_[:, w, :, 1, 1:257],
                                in0=qt[:, si, w, :, 2:258],
                                in1=xst[:, si, w, :, 1:257], op=Alu.add)

        # ------- window duplication (Wa c62..189, Wb c126..253), 4 queues --
        for (src, dst), q1, q2 in ((((Ag, AgW)), nc.gpsimd, nc.sync),
                                   (((Bg, BgW)), nc.scalar, nc.gpsimd)):
            q1.dma_start(out=dst[0:66, 0], in_=src[62:128, 0])
            q2.dma_start(out=dst[66:128, 0], in_=src[0:62, 1])
            q1.dma_start(out=dst[0:2, 1], in_=src[126:128, 0])
            q2.dma_start(out=dst[2:128, 1], in_=src[0:126, 1])

        movs_A = (Ag, AgW, AgW, Ag)
        movs_B = (Bg, BgW, BgW, Bg)
        wsl = (0, 0, 1, 1)   # window slot within the mov tile, by real t
        # physical slot p in the out tile -> real chunk t; pairs (t0,t3) and
        # (t1,t2) so the first psum tile of each image has no window deps
        TP = (0, 3, 1, 2)
        negI = stats[:, 16 * 128:17 * 128]

        # mov slices per (dy, out-parity): out i=2n+par needs A'[2n+par+dy]
        # dy=-1: E<-o[0:256], O<-e[0:256]; dy=0: E<-e[0:256], O<-o[1:257];
        # dy=+1: E<-o[1:257],  O<-e[1:257]
        MOVSL = {(-1, 0): (1, 0, 256), (-1, 1): (0, 0, 256),
                 (0, 0): (0, 0, 256), (0, 1): (1, 1, 257),
                 (1, 0): (1, 1, 257), (1, 1): (0, 1, 257)}

        def conv_img(m, half):
            pre = ppre.tile([128, 2, 512], F32, name="pre", tag="pre")
            for ti in range(2):
                t = TP[2 * half + ti]
                mv = movs_A[t]
                for par in range(2):          # psum cols [0:256]=even i
                    for dyi, dy in enumerate((-1, 0, 1)):
                        pp_, lo, hi = MOVSL[(dy, par)]
                        nc.tensor.matmul(
                            pre[:, ti, 256 * par:256 * par + 256],
                            stats[:, (3 * t + dyi) * 128:(3 * t + dyi + 1) * 128],
                            mv[:, wsl[t], m, pp_, lo:hi],
                            start=(dyi == 0), stop=(dyi == 2))
            return pre

        def g_img(m, half, movs, d_slot, use_act):
            gp = pg.tile([128, 2, 512], F32, name="gp", tag="g")
            for ti in range(2):
                p = 2 * half + ti
                t = TP[p]
                for par in range(2):
                    pp_, lo, hi = MOVSL[(0, par)]
                    cols = slice(256 * par, 256 * par + 256)
                    nc.tensor.matmul(gp[:, ti, cols],
                                     stats[:, (12 + t) * 128:(13 + t) * 128],
                                     movs[t][:, wsl[t], m, pp_, lo:hi],
                                     start=True, stop=False)
                    nc.tensor.matmul(gp[:, ti, cols], negI,
                                     out[:, m, p, cols],
                                     start=False, stop=True)
            # psum holds g - out; ACT Square + accum
            dtile = deadA if use_act else deadV
            sc.activation(out=dtile[:, :, :], in_=gp[:, :, :],
                          func=Act.Square,
                          accum_out=acc[:, d_slot:d_slot + 1])

        DQ = (nc.sync, nc.scalar, nc.gpsimd, nc.sync)

        for m in range(IMG):
            for half in range(2):
                pre = conv_img(m, half)
                sc.activation(out=out[:, m, 2 * half:2 * half + 2, :],
                              in_=pre[:, :, :], func=Act.Relu)
                v.tensor_scalar(out=out[:, m, 2 * half:2 * half + 2, :],
                                in0=out[:, m, 2 * half:2 * half + 2, :],
                                scalar1=1.0, scalar2=None, op0=Alu.min)
                g_img(m, half, movs_A, m * 2 + half, True)
                g_img(m, half, movs_B, 8 + m * 2 + half, True)
            # wavelet phase shuffles for img m; u = 64t + brev6(q):
            # Ee th slot 0 <- real t0 (p=0), th 1 <- real t2 (p=3);
            # Ee lanes 64: <- odd real t (t1 -> p=2, t3 -> p=1)
            for k, (dst, pl_, ph, psl) in enumerate((
                    (Ee, slice(0, 64), 0, 0), (Ee, slice(0, 64), 1, 3),
                    (Ee, slice(64, 128), 0, 2), (Ee, slice(64, 128), 1, 1),
                    (Oo, slice(0, 64), 0, 0), (Oo, slice(0, 64), 1, 3),
                    (Oo, slice(64, 128), 0, 2), (Oo, slice(64, 128), 1, 1))):
                src = slice(0, 64) if dst is Ee else slice(64, 128)
                DQ[(m + k) % 4].dma_start(out=dst[pl_, m, ph, :],
                                          in_=out[src, m, psl, :])
            # per-image wavelet level 1; i-dim inside out/Ee/Oo is
            # parity-split [e(256)|o(256)] so row pairs are the two halves
            v.tensor_tensor(out=sw[:, m, :, :], in0=Ee[:, m, :, :],
                            in1=Oo[:, m, :, :], op=Alu.add)
            v.tensor_tensor(out=dw[:, m, :, :], in0=Ee[:, m, :, :],
                            in1=Oo[:, m, :, :], op=Alu.subtract)
            swv = sw[:, m, :, :].rearrange("p b (h k) -> p b h k", h=2)
            dwv = dw[:, m, :, :].rearrange("p b (h k) -> p b h k", h=2)
            v.tensor_tensor(out=ll1[:, m, :, :], in0=swv[:, :, 0, :],
                            in1=swv[:, :, 1, :], op=Alu.add)
            v.tensor_tensor(out=lh1[:, m, :, :], in0=dwv[:, :, 0, :],
                            in1=dwv[:, :, 1, :], op=Alu.add)
            v.tensor_tensor(out=hl1[:, m, :, :], in0=swv[:, :, 0, :],
                            in1=swv[:, :, 1, :], op=Alu.subtract)
            v.tensor_tensor(out=hh1[:, m, :, :], in0=dwv[:, :, 0, :],
                            in1=dwv[:, :, 1, :], op=Alu.subtract)
            for k, det in enumerate((lh1, hl1, hh1)):
                sc.activation(out=det[:, m, :, :], in_=det[:, m, :, :],
                              func=Act.Abs)
                v.tensor_scalar(out=det[:, m, :, :], in0=det[:, m, :, :],
                                scalar1=THR[0], scalar2=None,
                                op0=Alu.min, op1=Alu.add,
                                accum_out=acc[:, 16 + 3 * m + k:17 + 3 * m + k])

        # lvl2 shuffles (bit-reversed lanes -> contiguous ranges):
        # E2[32*tv + s] <- ll1[(tv&1)*64 + s, :, tv>>1, :]      (s < 32)
        # O2[32*tv + s] <- ll1[(tv&1)*64 + 32 + s, :, tv>>1, :]
        for tv in range(4):
            sb = (tv & 1) * 64
            th = tv >> 1
            DQ[tv].dma_start(out=E2[32 * tv:32 * tv + 32, :, :],
                             in_=ll1[sb:sb + 32, :, th, :])
            DQ[(tv + 1) % 4].dma_start(out=O2[32 * tv:32 * tv + 32, :, :],
                                       in_=ll1[sb + 32:sb + 64, :, th, :])
        v.tensor_tensor(out=sw2[:, :, :], in0=E2[:, :, :], in1=O2[:, :, :],
                        op=Alu.add)
        v.tensor_tensor(out=dw2[:, :, :], in0=E2[:, :, :], in1=O2[:, :, :],
                        op=Alu.subtract)
        s2r = sw2[:, :, :].rearrange("p a (k two) -> p a k two", two=2)
        d2r = dw2[:, :, :].rearrange("p a (k two) -> p a k two", two=2)
        v.tensor_tensor(out=ll2[:, :, :], in0=s2r[..., 0], in1=s2r[..., 1],
                        op=Alu.add)
        v.tensor_tensor(out=lh2[:, :, :], in0=d2r[..., 0], in1=d2r[..., 1],
                        op=Alu.add)
        v.tensor_tensor(out=hl2[:, :, :], in0=s2r[..., 0], in1=s2r[..., 1],
                        op=Alu.subtract)
        v.tensor_tensor(out=hh2[:, :, :], in0=d2r[..., 0], in1=d2r[..., 1],
                        op=Alu.subtract)
        for k, det in enumerate((lh2, hl2, hh2)):
            sc.activation(out=det[:, :, :], in_=det[:, :, :], func=Act.Abs)
            v.tensor_scalar(out=det[:, :, :], in0=det[:, :, :],
                            scalar1=THR[1], scalar2=None,
                            op0=Alu.min, op1=Alu.add,
                            accum_out=acc[:, 28 + k:29 + k])

        # lvl3: E3[16*tv + r] <- ll2[32*tv + r] (r < 16), O3 <- +16
        for tv in range(4):
            DQ[tv].dma_start(out=E3[16 * tv:16 * tv + 16, :, :],
                             in_=ll2[32 * tv:32 * tv + 16, :, :])
            DQ[(tv + 1) % 4].dma_start(out=O3[16 * tv:16 * tv + 16, :, :],
                                       in_=ll2[32 * tv + 16:32 * tv + 32, :, :])
        v.tensor_tensor(out=sw3[0:64, :, :], in0=E3[0:64, :, :],
                        in1=O3[0:64, :, :], op=Alu.add)
        v.tensor_tensor(out=dw3[0:64, :, :], in0=E3[0:64, :, :],
                        in1=O3[0:64, :, :], op=Alu.subtract)
        s3r = sw3[:, :, :].rearrange("p a (k two) -> p a k two", two=2)
        d3r = dw3[:, :, :].rearrange("p a (k two) -> p a k two", two=2)
        v.tensor_tensor(out=lh3[0:64, :, :], in0=d3r[0:64, :, :, 0],
                        in1=d3r[0:64, :, :, 1], op=Alu.add)
        v.tensor_tensor(out=hl3[0:64, :, :], in0=s3r[0:64, :, :, 0],
                        in1=s3r[0:64, :, :, 1], op=Alu.subtract)
        v.tensor_tensor(out=hh3[0:64, :, :], in0=d3r[0:64, :, :, 0],
                        in1=d3r[0:64, :, :, 1], op=Alu.subtract)
        for k, det in enumerate((lh3, hl3, hh3)):
            sc.activation(out=det[0:64, :, :], in_=det[0:64, :, :],
                          func=Act.Abs)
            v.tensor_scalar(out=det[0:64, :, :], in0=det[0:64, :, :],
                            scalar1=THR[2], scalar2=None,
                            op0=Alu.min, op1=Alu.add,
                            accum_out=acc[0:64, 31 + k:32 + k])

        nc.gpsimd.dma_start(out=outh.ap(), in_=acc[:, :])

    import os
    if not os.environ.get("SKIP_WAIT_SPLIT"):
        _split_multiwaits(nc, mybir)
    return nc


def _split_multiwaits(nc, mybir):
    """HW instructions support exactly ONE sync-wait; split extras into
    standalone Drains (same post-pass as the previous kernel)."""
    for f in nc.m.functions:
        for bb in f.blocks:
            i = 0
            while i < len(bb.instructions):
                ins = bb.instructions[i]
                si = getattr(ins, "sync_info", None)
                if si is not None and si.on_wait and len(si.on_wait) > 1:
                    waits = list(si.on_wait)
                    for w in waits[:-1]:
                        d = mybir.InstDrain(
                            name=nc.get_next_instruction_name(),
                            ins=[], outs=[], bass_is_fusable=False)
                        d.engine = ins.engine
                        d.sync_info = mybir.SyncInfo(on_wait=[w], on_update=[])
                        bb.instructions.insert(i, d)
                        i += 1
                    ins.sync_info = mybir.SyncInfo(
                        on_wait=[waits[-1]], on_update=list(si.on_update))
                i += 1


def _get_nc():
    if "nc" not in _CACHE:
        _CACHE["nc"] = _build()
    return _CACHE["nc"]


def make_in_maps(noisy_input, weight):
    x = np.asarray(noisy_input, dtype=np.float32).reshape(B_TOTAL, H, W)
    stats = _build_stats(weight)
    maps = []
    for c in range(N_CORES):
        xs = np.zeros((128, 2, 2, IMG, 258), dtype=np.float16)
        for m in range(IMG):
            img = x[c * IMG + m]
            for s, ph in enumerate((img[0::2, 0::2], img[1::2, 1::2])):
                pt = np.ascontiguousarray(ph.T).astype(np.float16)  # [c, r]
                st = np.concatenate([pt[:, :1], pt, pt[:, -1:]], axis=1)
                xs[:, s, 0, m, :] = st[0:128]
                xs[:, s, 1, m, :] = st[128:256]
        maps.append({"xs": xs, "st": stats})
    return maps


def _host_combine(parts):
    d1 = d2 = 0.0
    wav = np.zeros(3)
    for p in parts:
        q = p.astype(np.float64)
        d1 += q[:, 0:8].sum()
        d2 += q[:, 8:16].sum()
        wav[0] += q[:, 16:28].sum()
        wav[1] += q[:, 28:31].sum()
        wav[2] += q[0:64, 31:34].sum()
    N = B_TOTAL * H * W
    reg = d1 / N
    rec = d2 / N
    wtot = 0.0
    for j in (1, 2, 3):
        lvl = 3 - j + 1
        Nj = B_TOTAL * (H // 2 ** j) ** 2 * 3
        wtot += (1.0 / lvl) * (wav[j - 1] / (2.0 ** j)) / Nj
    return np.float32(rec + GAMMA * reg + WAVELET_WEIGHT * wtot)


def kernel(noisy_input, weight):
    from concourse.bass_utils import run_bass_kernel_spmd
    nc = _get_nc()
    in_maps = make_in_maps(noisy_input, weight)
    res = run_bass_kernel_spmd(nc, in_maps, list(range(N_CORES)))
    return _host_combine([r["res"] for r in res.results])
